# revision 26
# baseline (speedup 1.0000x reference)
"""CAM (channel attention) kernel for Trainium2, SPMD over 8 NeuronCores.

Full inputs: x [16, 512, 64, 64] f32, gamma [1] f32.
Math per batch b (N = 64*64 = 4096 pixels, C = 512 channels):
    q = x[b].reshape(C, N)
    E = q @ q.T                            # (C, C)
    A = softmax(rowmax(E) - E, axis=-1)    # == exp(rowmin(E) - E) / rowsum
    y[b] = gamma * (A @ q) + x[b]

Sharding: data-parallel over batch, 2 of 16 batch elements per core.

Host-side prep (part of input sharding): x is cast to bf16/fp8 and laid
out three ways — natural [C, N] bf16 (residual), pixel-major q^T bf16
tiles (E-matmul operands), and natural fp8 (out-matmul rhs). This
removes all on-device transposes and casts. y is produced in bf16 and
upcast to f32 on the host (~0.2% rms rounding, budget 2e-2).

E is symmetric: only upper-triangle row segments are computed (row m
covers columns m*128..511); lower blocks are filled by PE transposes.
Softmax stats (min, exp) read E straight from PSUM; only the slice of
each row needed by later transpose-fills is copied to SBUF. The out
matmul A @ q runs in fp8 DoubleRow (K=256) in 512-px PSUM groups from
a single 8-buf PSUM pool so the PE never waits on evacuation, which is
split three ways: Vector direct adds, Scalar copy + GpSimd add, Scalar
copy + Vector add (alternating by row so neither V nor G exceeds the
16x216ns/row PE pace). A^T blocks for both batches are built before
out0, with zero-weight keepalive matmuls holding the HAM clock-gate at
8/8 through the transpose-only window (PE transposes don't register as
HAM activity). y rows alternate between the scalar (ACT) and sync (SP)
HWDGE rings — the ~170 GB/s HBM write rate is partly per-ring, so two
rings drain ~250 GB/s — and the final row goes out in 1024-px chunks
so the last write is small. Phases stay load/store-separated in time
(E0, E1, A^T, out0, out1): all 21MB of loads complete before the 8.4MB
write stream starts, so reads and writes never fight for HBM. The
measured end-of-kernel teardown (full semaphore-range reset, ~7us) and
runtime preamble are fixed costs; the y-write drain hides under the
teardown.
"""

from contextlib import ExitStack

import numpy as np
import ml_dtypes

import concourse.bacc as bacc
import concourse.bass as bass
import concourse.mybir as mybir
import concourse.tile as tile
from concourse.bass_utils import run_bass_kernel_spmd
from concourse.masks import make_identity

P = 128            # SBUF partitions
C = 512            # channels
CT = C // P        # 4 channel chunks
NPIX = 4096        # H*W
SL = 512           # pixel-slice width (one PSUM bank of f32)
NS = NPIX // SL    # 8 pixel slices
KT = NPIX // P     # 32 contraction chunks for E
MB = 2             # batch elements per core
NCORES = 8
B = 16

# qt load chunk sizes (in 128-pixel k-chunks); batch 0 front-loads small
# chunks so the first E matmul starts as early as possible
QT_SIZES = [[2, 2, 4, 4, 4, 4, 4, 4, 4], [4] * 8]

F32 = mybir.dt.float32
BF16 = mybir.dt.bfloat16
FP8 = mybir.dt.float8e4
BF16NP = ml_dtypes.bfloat16
FP8NP = ml_dtypes.float8_e4m3
AX = mybir.AxisListType.X
MIN = mybir.AluOpType.min
EXP = mybir.ActivationFunctionType.Exp
COPY = mybir.ActivationFunctionType.Copy
DROW = mybir.MatmulPerfMode.DoubleRow

# (m, j) lower-triangle blocks, grouped by source row j so fills become
# ready in emission order right after E ends
FILLS = [(1, 0), (2, 0), (3, 0), (2, 1), (3, 1), (3, 2)]


def build_nc() -> bacc.Bacc:
    nc = bacc.Bacc("TRN2", target_bir_lowering=False, debug=False)
    # xt[b, p, k, c] = bf16 x[b, c, 128k+p]  (q^T tiles)
    xt = nc.declare_dram_parameter("xt", [MB, P, KT, C], BF16, isOutput=False)
    # xb[b, p, t, n] = bf16 x[b, 128t+p, n]  (natural, partition-interleaved)
    xb = nc.declare_dram_parameter("xb", [MB, P, CT, NPIX], BF16, isOutput=False)
    # xq[b, p, t, n] = fp8 x[b, 128t+p, n]  (out-matmul rhs)
    xq = nc.declare_dram_parameter("xq", [MB, P, CT, NPIX], FP8, isOutput=False)
    g = nc.declare_dram_parameter("gamma", [1], F32, isOutput=False)
    y = nc.declare_dram_parameter("y", [MB, CT, P, NPIX], BF16, isOutput=True)

    xtv, xbv, xqv, yv = xt[:], xb[:], xq[:], y[:]

    with tile.TileContext(nc) as tc, ExitStack() as ctx:
        qtheadp = ctx.enter_context(tc.tile_pool(name="qth", bufs=2))
        qtpool = ctx.enter_context(tc.tile_pool(name="qt", bufs=12))
        qbpool = ctx.enter_context(tc.tile_pool(name="qb", bufs=MB))
        ypool = ctx.enter_context(tc.tile_pool(name="y", bufs=4))
        obpool = ctx.enter_context(tc.tile_pool(name="ob", bufs=3))
        q8pool = ctx.enter_context(tc.tile_pool(name="q8", bufs=MB))
        ecpool = ctx.enter_context(tc.tile_pool(name="ec", bufs=4))
        atpool = ctx.enter_context(tc.tile_pool(name="at", bufs=2))
        apool = ctx.enter_context(tc.tile_pool(name="a", bufs=4))
        upool = ctx.enter_context(tc.tile_pool(name="u", bufs=3))
        stat = ctx.enter_context(tc.tile_pool(name="stat", bufs=8))
        cpool = ctx.enter_context(tc.tile_pool(name="const", bufs=1))
        # 8 slots x 1 PSUM bank: E rows (4 alive), out groups (deep
        # rotation), A^T transposes, warm-up — all rotate through one pool
        psum = ctx.enter_context(tc.tile_pool(name="psum", bufs=8, space="PSUM"))

        # HAM pre-warm: dummy matmuls during the load ramp keep the PE
        # clock-gate at 8/8 so E0's first real matmuls run at 2.4 GHz.
        wz = cpool.tile([P, P], BF16)
        nc.vector.memset(wz[:], 0)
        wr = upool.tile([P, SL], BF16, tag="u", name="wr")
        nc.vector.memset(wr[:], 0)
        wr2 = cpool.tile([P, SL], BF16)
        nc.vector.memset(wr2[:], 0)
        warm = psum.tile([P, SL], F32, tag="ps", name="warm")
        for _ in range(4):
            nc.tensor.matmul(warm[:], wz[:], wr[:], start=True,
                             stop=True)
        ident = cpool.tile([P, P], BF16)
        make_identity(nc, ident[:])
        gamma_b = cpool.tile([P, 1], F32)
        nc.gpsimd.dma_start(gamma_b[:], g[:].to_broadcast((P, 1)))
        ident32 = cpool.tile([P, P], F32)
        make_identity(nc, ident32[:])

        # ---- issue all loads up front (HWDGE FIFO: in order of need) ----
        # kmap[b][k] = (tile, local k index)
        kmap = [[None] * KT for _ in range(MB)]
        qb8 = {}
        qb = {}

        def load_qt(b):
            k0 = 0
            for j, sz in enumerate(QT_SIZES[b]):
                pool = qtheadp if sz < 4 else qtpool
                t_ = pool.tile([P, sz, C], BF16, tag=f"qt{sz}",
                               name=f"qt{b}_{j}")
                nc.sync.dma_start(t_[:], xtv[b, :, k0:k0 + sz, :])
                for kk in range(sz):
                    kmap[b][k0 + kk] = (t_, kk)
                k0 += sz

        def load_q8(b):
            t8 = q8pool.tile([P, CT, NPIX], FP8, tag="q8", name=f"q8{b}")
            nc.sync.dma_start(t8[:], xqv[b])
            qb8[b] = t8

        def load_qb(b):
            t_ = qbpool.tile([P, CT, NPIX], BF16, tag="qb", name=f"qb{b}")
            nc.sync.dma_start(t_[:], xbv[b])
            qb[b] = t_

        load_qt(0)
        load_q8(0)
        load_qt(1)
        load_qb(0)
        load_q8(1)
        load_qb(1)

        e_ps = {}
        ecp = {}

        def e_sm_phase(b, m_order):
            """E accumulation in two halves: k<KH interleaved over rows,
            k>=KH row-contiguous so each row finishes early and its
            softmax chain pipelines under the remaining E matmuls.
            Softmax stats read E from PSUM; only the row slice needed by
            later transpose-fills is copied to SBUF."""
            KH = KT // 2
            tiles = {}
            for m in m_order:
                tiles[m] = psum.tile([P, C], F32, tag="ps", name=f"e{b}_{m}")
            e_ps[b] = tiles
            ecp[b] = {}
            a_t = {}
            for k in range(KH):
                src, kk = kmap[b][k]
                for m in m_order:
                    nc.tensor.matmul(
                        e_ps[b][m][:, m * P:],
                        src[:, kk, m * P:(m + 1) * P],
                        src[:, kk, m * P:],
                        start=(k == 0),
                        stop=False,
                    )
            for m in range(CT):
                for k in range(KH, KT):
                    src, kk = kmap[b][k]
                    nc.tensor.matmul(
                        e_ps[b][m][:, m * P:],
                        src[:, kk, m * P:(m + 1) * P],
                        src[:, kk, m * P:],
                        start=False,
                        stop=(k == KT - 1),
                    )
                for j in range(m):
                    nc.tensor.transpose(
                        e_ps[b][m][:, j * P:(j + 1) * P],
                        ecp[b][j][:, m * P:(m + 1) * P],
                        ident32[:],
                    )
                if m < CT - 1:
                    ec = ecpool.tile([P, C], F32, tag="ec", name=f"ec{b}_{m}")
                    nc.scalar.copy(ec[:, (m + 1) * P:],
                                   e_ps[b][m][:, (m + 1) * P:])
                    ecp[b][m] = ec
                mn = stat.tile([P, 1], F32, tag="mn", name="mn")
                nc.vector.tensor_reduce(mn[:], e_ps[b][m][:], AX, MIN)
                u = upool.tile([P, C], F32, tag="u", name="u")
                sm = stat.tile([P, 1], F32, tag="sm", name="sm")
                nc.scalar.activation(
                    u[:], e_ps[b][m][:], EXP, bias=mn[:], scale=-1.0,
                    accum_out=sm[:]
                )
                rc = stat.tile([P, 1], F32, tag="rc", name="rc")
                nc.vector.reciprocal(rc[:], sm[:])
                a = apool.tile([P, C], BF16, tag="a", name=f"a{b}_{m}")
                nc.vector.tensor_scalar(
                    a[:], u[:], rc[:], gamma_b[:],
                    op0=mybir.AluOpType.mult, op1=mybir.AluOpType.mult,
                )
                a_t[m] = a
            return a_t


        def at_alloc(b):
            return atpool.tile([P, CT, C], FP8, tag="at", name=f"at{b}")

        def at_block(b, a_t, at_sb, m, cast_eng):
            """One m-block of A^T via PE into at_sb (fp8 lhsT layout)."""
            tp = psum.tile([P, C], BF16, tag="ps", name=f"atp{b}_{m}")
            for kk in range(CT):
                nc.tensor.transpose(
                    tp[:, kk * P:(kk + 1) * P],
                    a_t[m][:, kk * P:(kk + 1) * P],
                    ident[:],
                )
            if cast_eng == "v":
                nc.vector.tensor_copy(at_sb[:, :, m * P:(m + 1) * P], tp[:])
            else:
                nc.scalar.copy(at_sb[:, :, m * P:(m + 1) * P], tp[:])

        def out_phase(b, at_sb, extra=None):
            last = b == MB - 1
            for m in range(CT):
                lastrow = last and m == CT - 1
                ych = ypool.tile([P, NPIX], BF16, tag="y", name=f"y{b}_{m}")
                for ns in range(NS):
                    ops = psum.tile([P, SL], F32, tag="ps",
                                    name=f"o{b}_{m}_{ns}")
                    for gg in range(CT // 2):
                        nc.tensor.matmul(
                            ops[:],
                            at_sb[:, 2 * gg:2 * gg + 2, m * P:(m + 1) * P],
                            qb8[b][:, 2 * gg:2 * gg + 2,
                                   ns * SL:(ns + 1) * SL],
                            start=(gg == 0),
                            stop=(gg == CT // 2 - 1),
                            perf_mode=DROW,
                        )
                    res = qb[b][:, m, ns * SL:(ns + 1) * SL]
                    yslice = ych[:, ns * SL:(ns + 1) * SL]
                    # 3-way PSUM evacuation: Vector direct (even slices),
                    # Scalar copy + GpSimd add (1,3,5), Scalar copy +
                    # Vector bf16 add (7)
                    if ns % 2 == 0:
                        nc.vector.tensor_add(yslice, ops[:], res)
                    else:
                        ob = obpool.tile([P, SL], BF16, tag="ob", name="ob")
                        nc.scalar.copy(ob[:], ops[:])
                        if ns == NS - 1 or (ns == 5 and m % 2 == 1):
                            nc.vector.tensor_add(yslice, ob[:], res)
                        else:
                            nc.gpsimd.tensor_add(yslice, ob[:], res)
                    if lastrow and ns % 2 == 1:
                        eng = nc.scalar if ns % 4 == 1 else nc.sync
                        eng.dma_start(
                            yv[b, m, :, (ns - 1) * SL:(ns + 1) * SL],
                            ych[:, (ns - 1) * SL:(ns + 1) * SL],
                        )
                if not lastrow:
                    # alternate HWDGE rings: the ~172 GB/s write rate is
                    # partly per-ring; two rings raise the drain rate
                    eng = nc.scalar if m % 2 == 0 else nc.sync
                    eng.dma_start(yv[b, m], ych[:])
                if extra is not None:
                    extra(m)

        def ham_keepalive(n):
            # transpose-mode PE work does not register as HAM activity;
            # zero matmuls through the at-block windows keep K=8/8 so
            # the out phases start at full clock
            w = psum.tile([P, SL], F32, tag="ps", name="keep")
            for _ in range(n):
                nc.tensor.matmul(w[:], wz[:], wr2[:], start=True, stop=True)

        # phase order E0, E1, at, out0, out1: loads (through ~57us)
        # finish before the y-write stream begins (~60us), so reads and
        # writes never fight for HBM bandwidth
        a0 = e_sm_phase(0, [0, 1, 2, 3])
        a1 = e_sm_phase(1, [3, 2, 1, 0])
        at0 = at_alloc(0)
        for m in range(CT):
            at_block(0, a0, at0, m, "v" if m % 2 == 0 else "s")
            ham_keepalive(1)
        at1 = at_alloc(1)
        for m in range(CT):
            at_block(1, a1, at1, m, "v" if m % 2 == 1 else "s")
            ham_keepalive(1)
        out_phase(0, at0)
        out_phase(1, at1)

    return nc


_NC = None


def _get_nc() -> bacc.Bacc:
    global _NC
    if _NC is None:
        _NC = build_nc()
        _NC.finalize()
    return _NC


def _prep(x: np.ndarray):
    """Cast to bf16/fp8 and lay out the three tile forms."""
    xr = np.ascontiguousarray(x, dtype=np.float32).reshape(B, C, NPIX)
    x16 = xr.astype(BF16NP)
    xb_t = np.ascontiguousarray(
        x16.reshape(B, CT, P, NPIX).transpose(0, 2, 1, 3)  # [B, P, CT, NPIX]
    )
    xt_t = np.ascontiguousarray(
        x16.reshape(B, C, KT, P).transpose(0, 3, 2, 1)       # [B, P, KT, C]
    )
    xq_t = np.ascontiguousarray(
        x16.astype(FP8NP).reshape(B, CT, P, NPIX).transpose(0, 2, 1, 3)
    )                                                        # [B, P, CT, NPIX]
    return xb_t, xt_t, xq_t


def _run(x: np.ndarray, gamma: np.ndarray, trace: bool = False):
    gamma = np.ascontiguousarray(gamma, dtype=np.float32).reshape(1)
    xb_t, xt_t, xq_t = _prep(x)
    in_maps = [
        {
            "xt": xt_t[MB * i:MB * (i + 1)],
            "xb": xb_t[MB * i:MB * (i + 1)],
            "xq": xq_t[MB * i:MB * (i + 1)],
            "gamma": gamma,
        }
        for i in range(NCORES)
    ]
    res = run_bass_kernel_spmd(
        _get_nc(), in_maps, core_ids=list(range(NCORES)), trace=trace
    )
    out = np.concatenate(
        [np.asarray(r["y"], dtype=np.float32) for r in res.results], axis=0
    )
    out = out.reshape(B, C, 64, 64)
    return out.astype(np.float32, copy=False), res


def kernel(x: np.ndarray, gamma: np.ndarray) -> np.ndarray:
    out, _ = _run(x, gamma, trace=False)
    return out


def kernel_profiled(x: np.ndarray, gamma: np.ndarray):
    out, res = _run(x, gamma, trace=True)
    return out, res


# revision 27
# speedup vs baseline: 1.1446x; 1.1446x over previous
"""CAM (channel attention) kernel for Trainium2, SPMD over 8 NeuronCores.

Full inputs: x [16, 512, 64, 64] f32, gamma [1] f32.
Math per batch b (N = 64*64 = 4096 pixels, C = 512 channels):
    q = x[b].reshape(C, N)
    E = q @ q.T                            # (C, C)
    A = softmax(rowmax(E) - E, axis=-1)    # == exp(rowmin(E) - E) / rowsum
    y[b] = gamma * (A @ q) + x[b]

Sharding: data-parallel over batch, 2 of 16 batch elements per core.

Host-side prep (part of input sharding): x is cast to bf16/fp8 and laid
out three ways — natural [C, N] bf16 (residual), pixel-major q^T bf16
tiles (E-matmul operands), and natural fp8 (out-matmul rhs). This
removes all on-device transposes and casts. y is produced in bf16 and
upcast to f32 on the host (~0.2% rms rounding, budget 2e-2).

E is symmetric: only upper-triangle row segments are computed (row m
covers columns m*128..511); lower blocks are filled by PE transposes.
Softmax stats (min, exp) read E straight from PSUM; only the slice of
each row needed by later transpose-fills is copied to SBUF. The out
matmul A @ q runs in fp8 DoubleRow (K=256) in 512-px PSUM groups from
a single 8-buf PSUM pool so the PE never waits on evacuation, which is
split three ways: Vector direct adds, Scalar copy + GpSimd add, Scalar
copy + Vector add (alternating by row so neither V nor G exceeds the
16x216ns/row PE pace). A^T blocks for both batches are built before
out0, with zero-weight keepalive matmuls holding the HAM clock-gate at
8/8 through the transpose-only window (PE transposes don't register as
HAM activity). y rows alternate between the scalar (ACT) and sync (SP)
HWDGE rings — the ~170 GB/s HBM write rate is partly per-ring, so two
rings drain ~250 GB/s — and the final row goes out in 1024-px chunks
so the last write is small. Phases stay load/store-separated in time
(E0, E1, A^T, out0, out1): all 21MB of loads complete before the 8.4MB
write stream starts, so reads and writes never fight for HBM. The
measured end-of-kernel teardown (full semaphore-range reset, ~7us) and
runtime preamble are fixed costs; the y-write drain hides under the
teardown.
"""

from contextlib import ExitStack

import numpy as np
import ml_dtypes

import concourse.bacc as bacc
import concourse.bass as bass
import concourse.mybir as mybir
import concourse.tile as tile
from concourse.bass_utils import run_bass_kernel_spmd
from concourse.masks import make_identity

P = 128            # SBUF partitions
C = 512            # channels
CT = C // P        # 4 channel chunks
NPIX = 4096        # H*W
SL = 512           # pixel-slice width (one PSUM bank of f32)
NS = NPIX // SL    # 8 pixel slices
KT = NPIX // P     # 32 contraction chunks for E
MB = 2             # batch elements per core
NCORES = 8
B = 16

# qt load chunk sizes (in 128-pixel k-chunks); batch 0 front-loads small
# chunks so the first E matmul starts as early as possible
QT_SIZES = [[2, 2, 4, 4, 4, 4, 4, 4, 4], [4] * 8]

F32 = mybir.dt.float32
BF16 = mybir.dt.bfloat16
FP8 = mybir.dt.float8e4
BF16NP = ml_dtypes.bfloat16
FP8NP = ml_dtypes.float8_e4m3
AX = mybir.AxisListType.X
MIN = mybir.AluOpType.min
EXP = mybir.ActivationFunctionType.Exp
COPY = mybir.ActivationFunctionType.Copy
DROW = mybir.MatmulPerfMode.DoubleRow

# (m, j) lower-triangle blocks, grouped by source row j so fills become
# ready in emission order right after E ends
FILLS = [(1, 0), (2, 0), (3, 0), (2, 1), (3, 1), (3, 2)]


def build_nc() -> bacc.Bacc:
    nc = bacc.Bacc("TRN2", target_bir_lowering=False, debug=False)
    # xt[b, p, k, c] = bf16 x[b, c, 128k+p]  (q^T tiles)
    xt = nc.declare_dram_parameter("xt", [MB, P, KT, C], BF16, isOutput=False)
    # xb[b, p, t, n] = bf16 x[b, 128t+p, n]  (natural, partition-interleaved)
    xb = nc.declare_dram_parameter("xb", [MB, P, CT, NPIX], BF16, isOutput=False)
    # xq[b, p, t, n] = fp8 x[b, 128t+p, n]  (out-matmul rhs)
    xq = nc.declare_dram_parameter("xq", [MB, P, CT, NPIX], FP8, isOutput=False)
    g = nc.declare_dram_parameter("gamma", [1], F32, isOutput=False)
    y = nc.declare_dram_parameter("y", [MB, CT, P, NPIX], BF16, isOutput=True)

    xtv, xbv, xqv, yv = xt[:], xb[:], xq[:], y[:]

    with tile.TileContext(nc) as tc, ExitStack() as ctx:
        qtheadp = ctx.enter_context(tc.tile_pool(name="qth", bufs=2))
        qtpool = ctx.enter_context(tc.tile_pool(name="qt", bufs=12))
        qbpool = ctx.enter_context(tc.tile_pool(name="qb", bufs=MB))
        ypool = ctx.enter_context(tc.tile_pool(name="y", bufs=4))
        obpool = ctx.enter_context(tc.tile_pool(name="ob", bufs=3))
        q8pool = ctx.enter_context(tc.tile_pool(name="q8", bufs=MB))
        ecpool = ctx.enter_context(tc.tile_pool(name="ec", bufs=4))
        atpool = ctx.enter_context(tc.tile_pool(name="at", bufs=2))
        apool = ctx.enter_context(tc.tile_pool(name="a", bufs=4))
        upool = ctx.enter_context(tc.tile_pool(name="u", bufs=3))
        stat = ctx.enter_context(tc.tile_pool(name="stat", bufs=8))
        cpool = ctx.enter_context(tc.tile_pool(name="const", bufs=1))
        # 8 slots x 1 PSUM bank: E rows (4 alive), out groups (deep
        # rotation), A^T transposes, warm-up — all rotate through one pool
        psum = ctx.enter_context(tc.tile_pool(name="psum", bufs=8, space="PSUM"))

        # HAM pre-warm: dummy matmuls during the load ramp keep the PE
        # clock-gate at 8/8 so E0's first real matmuls run at 2.4 GHz.
        wz = cpool.tile([P, P], BF16)
        nc.vector.memset(wz[:], 0)
        wr = upool.tile([P, SL], BF16, tag="u", name="wr")
        nc.vector.memset(wr[:], 0)
        wr2 = cpool.tile([P, SL], BF16)
        nc.vector.memset(wr2[:], 0)
        warm = psum.tile([P, SL], F32, tag="ps", name="warm")
        for _ in range(4):
            nc.tensor.matmul(warm[:], wz[:], wr[:], start=True,
                             stop=True)
        ident = cpool.tile([P, P], BF16)
        make_identity(nc, ident[:])
        gamma_b = cpool.tile([P, 1], F32)
        nc.gpsimd.dma_start(gamma_b[:], g[:].to_broadcast((P, 1)))
        ident32 = cpool.tile([P, P], F32)
        make_identity(nc, ident32[:])

        # ---- issue all loads up front (HWDGE FIFO: in order of need) ----
        # kmap[b][k] = (tile, local k index)
        kmap = [[None] * KT for _ in range(MB)]
        qb8 = {}
        qb = {}

        def load_qt(b):
            k0 = 0
            for j, sz in enumerate(QT_SIZES[b]):
                pool = qtheadp if sz < 4 else qtpool
                t_ = pool.tile([P, sz, C], BF16, tag=f"qt{sz}",
                               name=f"qt{b}_{j}")
                nc.sync.dma_start(t_[:], xtv[b, :, k0:k0 + sz, :])
                for kk in range(sz):
                    kmap[b][k0 + kk] = (t_, kk)
                k0 += sz

        def load_q8(b):
            t8 = q8pool.tile([P, CT, NPIX], FP8, tag="q8", name=f"q8{b}")
            nc.sync.dma_start(t8[:], xqv[b])
            qb8[b] = t8

        def load_qb(b):
            t_ = qbpool.tile([P, CT, NPIX], BF16, tag="qb", name=f"qb{b}")
            nc.sync.dma_start(t_[:], xbv[b])
            qb[b] = t_

        load_qt(0)
        load_q8(0)
        load_qt(1)
        load_qb(0)
        load_q8(1)
        load_qb(1)

        e_ps = {}
        ecp = {}

        def e_sm_phase(b, m_order):
            """E accumulation in two halves: k<KH interleaved over rows,
            k>=KH row-contiguous so each row finishes early and its
            softmax chain pipelines under the remaining E matmuls.
            Softmax stats read E from PSUM; only the row slice needed by
            later transpose-fills is copied to SBUF."""
            KH = KT // 2
            tiles = {}
            for m in m_order:
                tiles[m] = psum.tile([P, C], F32, tag="ps", name=f"e{b}_{m}")
            e_ps[b] = tiles
            ecp[b] = {}
            a_t = {}
            for k in range(KH):
                src, kk = kmap[b][k]
                # ping-pong the row order so consecutive matmuls at the
                # k boundary hit the same PSUM bank (halves bank switches
                # vs a fixed rotation) while still consuming each chunk
                # at the pace the DMA delivers it
                mo = m_order if k % 2 == 0 else m_order[::-1]
                for m in mo:
                    nc.tensor.matmul(
                        e_ps[b][m][:, m * P:],
                        src[:, kk, m * P:(m + 1) * P],
                        src[:, kk, m * P:],
                        start=(k == 0),
                        stop=False,
                    )
            for m in range(CT):
                for k in range(KH, KT):
                    src, kk = kmap[b][k]
                    nc.tensor.matmul(
                        e_ps[b][m][:, m * P:],
                        src[:, kk, m * P:(m + 1) * P],
                        src[:, kk, m * P:],
                        start=False,
                        stop=(k == KT - 1),
                    )
                for j in range(m):
                    nc.tensor.transpose(
                        e_ps[b][m][:, j * P:(j + 1) * P],
                        ecp[b][j][:, m * P:(m + 1) * P],
                        ident32[:],
                    )
                if m < CT - 1:
                    ec = ecpool.tile([P, C], F32, tag="ec", name=f"ec{b}_{m}")
                    nc.scalar.copy(ec[:, (m + 1) * P:],
                                   e_ps[b][m][:, (m + 1) * P:])
                    ecp[b][m] = ec
                mn = stat.tile([P, 1], F32, tag="mn", name="mn")
                nc.vector.tensor_reduce(mn[:], e_ps[b][m][:], AX, MIN)
                u = upool.tile([P, C], F32, tag="u", name="u")
                sm = stat.tile([P, 1], F32, tag="sm", name="sm")
                nc.scalar.activation(
                    u[:], e_ps[b][m][:], EXP, bias=mn[:], scale=-1.0,
                    accum_out=sm[:]
                )
                rc = stat.tile([P, 1], F32, tag="rc", name="rc")
                nc.vector.reciprocal(rc[:], sm[:])
                a = apool.tile([P, C], BF16, tag="a", name=f"a{b}_{m}")
                nc.vector.tensor_scalar(
                    a[:], u[:], rc[:], gamma_b[:],
                    op0=mybir.AluOpType.mult, op1=mybir.AluOpType.mult,
                )
                a_t[m] = a
            return a_t


        def at_alloc(b):
            return atpool.tile([P, CT, C], FP8, tag="at", name=f"at{b}")

        def at_block(b, a_t, at_sb, m, cast_eng):
            """One m-block of A^T via PE into at_sb (fp8 lhsT layout)."""
            tp = psum.tile([P, C], BF16, tag="ps", name=f"atp{b}_{m}")
            for kk in range(CT):
                nc.tensor.transpose(
                    tp[:, kk * P:(kk + 1) * P],
                    a_t[m][:, kk * P:(kk + 1) * P],
                    ident[:],
                )
            if cast_eng == "v":
                nc.vector.tensor_copy(at_sb[:, :, m * P:(m + 1) * P], tp[:])
            else:
                nc.scalar.copy(at_sb[:, :, m * P:(m + 1) * P], tp[:])

        def out_phase(b, at_sb, extra=None):
            last = b == MB - 1
            for m in range(CT):
                lastrow = last and m == CT - 1
                ych = ypool.tile([P, NPIX], BF16, tag="y", name=f"y{b}_{m}")
                for ns in range(NS):
                    ops = psum.tile([P, SL], F32, tag="ps",
                                    name=f"o{b}_{m}_{ns}")
                    for gg in range(CT // 2):
                        nc.tensor.matmul(
                            ops[:],
                            at_sb[:, 2 * gg:2 * gg + 2, m * P:(m + 1) * P],
                            qb8[b][:, 2 * gg:2 * gg + 2,
                                   ns * SL:(ns + 1) * SL],
                            start=(gg == 0),
                            stop=(gg == CT // 2 - 1),
                            perf_mode=DROW,
                        )
                    res = qb[b][:, m, ns * SL:(ns + 1) * SL]
                    yslice = ych[:, ns * SL:(ns + 1) * SL]
                    # 3-way PSUM evacuation: Vector direct (even slices),
                    # Scalar copy + GpSimd add (1,3,5), Scalar copy +
                    # Vector bf16 add (7)
                    if ns % 2 == 0:
                        nc.vector.tensor_add(yslice, ops[:], res)
                    else:
                        ob = obpool.tile([P, SL], BF16, tag="ob", name="ob")
                        nc.scalar.copy(ob[:], ops[:])
                        if ns == NS - 1 or (ns == 5 and m % 2 == 1):
                            nc.vector.tensor_add(yslice, ob[:], res)
                        else:
                            nc.gpsimd.tensor_add(yslice, ob[:], res)
                    if lastrow and ns % 2 == 1:
                        eng = nc.scalar if ns % 4 == 1 else nc.sync
                        eng.dma_start(
                            yv[b, m, :, (ns - 1) * SL:(ns + 1) * SL],
                            ych[:, (ns - 1) * SL:(ns + 1) * SL],
                        )
                if not lastrow:
                    # alternate HWDGE rings: the ~172 GB/s write rate is
                    # partly per-ring; two rings raise the drain rate
                    eng = nc.scalar if m % 2 == 0 else nc.sync
                    eng.dma_start(yv[b, m], ych[:])
                if extra is not None:
                    extra(m)

        def ham_keepalive(n):
            # transpose-mode PE work does not register as HAM activity;
            # zero matmuls through the at-block windows keep K=8/8 so
            # the out phases start at full clock
            w = psum.tile([P, SL], F32, tag="ps", name="keep")
            for _ in range(n):
                nc.tensor.matmul(w[:], wz[:], wr2[:], start=True, stop=True)

        # phase order E0, E1, at, out0, out1: loads (through ~57us)
        # finish before the y-write stream begins (~60us), so reads and
        # writes never fight for HBM bandwidth
        a0 = e_sm_phase(0, [0, 1, 2, 3])
        a1 = e_sm_phase(1, [3, 2, 1, 0])
        at0 = at_alloc(0)
        for m in range(CT):
            at_block(0, a0, at0, m, "v" if m % 2 == 0 else "s")
            ham_keepalive(1)
        at1 = at_alloc(1)
        for m in range(CT):
            at_block(1, a1, at1, m, "v" if m % 2 == 1 else "s")
            ham_keepalive(1)
        out_phase(0, at0)
        out_phase(1, at1)

    return nc


_NC = None


def _get_nc() -> bacc.Bacc:
    global _NC
    if _NC is None:
        _NC = build_nc()
        _NC.finalize()
    return _NC


def _prep(x: np.ndarray):
    """Cast to bf16/fp8 and lay out the three tile forms."""
    xr = np.ascontiguousarray(x, dtype=np.float32).reshape(B, C, NPIX)
    x16 = xr.astype(BF16NP)
    xb_t = np.ascontiguousarray(
        x16.reshape(B, CT, P, NPIX).transpose(0, 2, 1, 3)  # [B, P, CT, NPIX]
    )
    xt_t = np.ascontiguousarray(
        x16.reshape(B, C, KT, P).transpose(0, 3, 2, 1)       # [B, P, KT, C]
    )
    xq_t = np.ascontiguousarray(
        x16.astype(FP8NP).reshape(B, CT, P, NPIX).transpose(0, 2, 1, 3)
    )                                                        # [B, P, CT, NPIX]
    return xb_t, xt_t, xq_t


def _run(x: np.ndarray, gamma: np.ndarray, trace: bool = False):
    gamma = np.ascontiguousarray(gamma, dtype=np.float32).reshape(1)
    xb_t, xt_t, xq_t = _prep(x)
    in_maps = [
        {
            "xt": xt_t[MB * i:MB * (i + 1)],
            "xb": xb_t[MB * i:MB * (i + 1)],
            "xq": xq_t[MB * i:MB * (i + 1)],
            "gamma": gamma,
        }
        for i in range(NCORES)
    ]
    res = run_bass_kernel_spmd(
        _get_nc(), in_maps, core_ids=list(range(NCORES)), trace=trace
    )
    out = np.concatenate(
        [np.asarray(r["y"], dtype=np.float32) for r in res.results], axis=0
    )
    out = out.reshape(B, C, 64, 64)
    return out.astype(np.float32, copy=False), res


def kernel(x: np.ndarray, gamma: np.ndarray) -> np.ndarray:
    out, _ = _run(x, gamma, trace=False)
    return out


def kernel_profiled(x: np.ndarray, gamma: np.ndarray):
    out, res = _run(x, gamma, trace=True)
    return out, res
